# revision 40
# baseline (speedup 1.0000x reference)
"""AttentionCritic Trainium2 kernel — full inputs in, full output out.

Data-parallel over batch B across 8 NeuronCores (Bs = B/8 = 4096 per core),
params replicated. Per core the forward runs as one Bass/Tile program in
feature-major fp16 layout (host pre-transposes inputs, packs weights).

Per 512-sample tile, three phases (grouped so the ScalarE activation
table switches at most ~3x per tile — table loads are ~1.3us each):
  E: per-agent encoders + K/V/S head projections (PE matmuls, ACT Lrelu,
     DVE PSUM evacuations)
  A: per-sample agent-attention: DVE pair products against a broadcast
     AP, PE block-ones reduce to expanded logits (1/sqrt(D) folded in),
     ACT exp (input shift -4 for fp16 range; self pair killed with a
     -25 shift instead of a mask), numerator and denominator via PE
     identity-matmul PSUM accumulation (denominator on DVE add-tree for
     odd agents to balance engines), DVE fast reciprocal + normalize
  C: per-agent critic MLP; q bias-add on DVE (keeps ACT on Lrelu only)
"""

import numpy as np

A = 8
B = 32768
SDIM = 128
ADIM = 32
H = 128
NH = 4
D = H // NH
M = 8            # cores
BS = B // M      # 4096 batch per core
BN = 512         # batch tile
NT = BS // BN    # 8 tiles


def _build_program():
    from contextlib import ExitStack
    import concourse.bacc as bacc
    import concourse.tile as tile
    import concourse.mybir as mybir

    F16 = mybir.dt.float16
    BF16 = mybir.dt.bfloat16
    F32 = mybir.dt.float32
    AF = mybir.ActivationFunctionType

    nc = bacc.Bacc("TRN2", target_bir_lowering=False, debug=False, num_devices=M)

    def din(name, shape, dt=F16):
        return nc.dram_tensor(name, shape, dt, kind="ExternalInput").ap()

    xs = din("xs", [A, SDIM, BS])              # states^T per agent
    xa = din("xa", [A, ADIM, BS])              # actions^T per agent
    w_enc_s = din("w_enc_s", [SDIM, A * H])
    w_enc_a = din("w_enc_a", [ADIM, A * H])
    w_senc = din("w_senc", [SDIM, A * H])
    w_key = din("w_key", [H, H])
    w_sel = din("w_sel", [H, H])
    w_val = din("w_val", [H, H])
    w_c1s = din("w_c1s", [H, A * H])
    w_c1o = din("w_c1o", [H, A * H])
    w_c2 = din("w_c2", [H, A * H])
    w_c3 = din("w_c3", [H, A])
    cR = din("cR", [H, H])                     # block-ones / sqrt(D)
    cI = din("cI", [H, H])                     # identity
    b_enc = din("b_enc", [H, A], F32)
    b_senc = din("b_senc", [H, A], F32)
    b_val = din("b_val", [H, 1], F32)
    b_c1 = din("b_c1", [H, A], F32)
    b_c2 = din("b_c2", [H, A], F32)
    b_c3 = din("b_c3", [1, A], F32)
    q = nc.dram_tensor("q", [A, BS], F32, kind="ExternalOutput").ap()

    with tile.TileContext(nc) as tc, ExitStack() as ctx:
        wp = ctx.enter_context(tc.tile_pool(name="w", bufs=1))
        io = ctx.enter_context(tc.tile_pool(name="io", bufs=2))
        io1 = ctx.enter_context(tc.tile_pool(name="io1", bufs=1))
        act = ctx.enter_context(tc.tile_pool(name="act", bufs=2))
        att = ctx.enter_context(tc.tile_pool(name="att", bufs=2))
        sm = ctx.enter_context(tc.tile_pool(name="sm", bufs=2))
        att3 = ctx.enter_context(tc.tile_pool(name="att3", bufs=3))
        ps = ctx.enter_context(tc.tile_pool(name="ps", bufs=1, space="PSUM"))
        lxp = ctx.enter_context(tc.tile_pool(name="lxp", bufs=2, space="PSUM"))
        nmp = ctx.enter_context(tc.tile_pool(name="nmp", bufs=2, space="PSUM"))
        zxp = ctx.enter_context(tc.tile_pool(name="zxp", bufs=1, space="PSUM"))

        def wtile(ap_, shape, dt=F16):
            # weight loads ride the gpsimd DMA queue so the first input
            # tile's sync-queue DMA isn't serialized behind them
            t = wp.tile(shape, dt, tag=ap_.name)
            nc.gpsimd.dma_start(t[:], ap_)
            return t

        t_enc_s = wtile(w_enc_s, [SDIM, A * H])
        t_enc_a = wtile(w_enc_a, [ADIM, A * H])
        t_senc = wtile(w_senc, [SDIM, A * H])
        t_key = wtile(w_key, [H, H])
        t_sel = wtile(w_sel, [H, H])
        t_val = wtile(w_val, [H, H])
        t_c1s = wtile(w_c1s, [H, A * H])
        t_c1o = wtile(w_c1o, [H, A * H])
        t_c2 = wtile(w_c2, [H, A * H])
        t_c3 = wtile(w_c3, [H, A])
        t_R = wtile(cR, [H, H], BF16)
        t_I = wtile(cI, [H, H])
        t_benc = wtile(b_enc, [H, A], F32)
        t_bsenc = wtile(b_senc, [H, A], F32)
        t_bval = wtile(b_val, [H, 1], F32)
        t_bc1 = wtile(b_c1, [H, A], F32)
        t_bc2 = wtile(b_c2, [H, A], F32)
        t_bc3 = wtile(b_c3, [1, A], F32)
        t_neg4 = wp.tile([H, 1], F32, tag="neg4")
        nc.gpsimd.memset(t_neg4[:], -4.0)
        t_neg25 = wp.tile([H, 1], F32, tag="neg25")
        nc.gpsimd.memset(t_neg25[:], -25.0)

        act_chain = []

        def sact(*args, **kw):
            act_chain.append(nc.scalar.activation(*args, **kw))

        for t in range(NT):
            b0 = t * BN
            # ---- input tiles: [feat, (agent, b)] ----
            xs_t = io.tile([SDIM, A * BN], F16, tag="xs")
            for g4 in range(4):
                ga = slice(2 * g4, 2 * g4 + 2)
                nc.sync.dma_start(
                    xs_t[:, 2 * g4 * BN:(2 * g4 + 2) * BN]
                    .rearrange("p (a b) -> p a b", a=2),
                    xs[ga, :, b0:b0 + BN].rearrange("a p b -> p a b"))
            xa_t = io1.tile([ADIM, A * BN], F16, tag="xa")
            nc.sync.dma_start(
                xa_t[:].rearrange("p (a b) -> p a b", a=A),
                xa[:, :, b0:b0 + BN].rearrange("a p b -> p a b"))

            sa_t = io1.tile([H, A * BN], F16, tag="sa")
            s_t = act.tile([H, A * BN], F16, tag="s")
            K_t = act.tile([H, A * BN], F16, tag="K")
            V_t = act.tile([H, A * BN], F16, tag="V")
            S_t = act.tile([H, A * BN], F16, tag="S")
            oth_t = act.tile([H, A * BN], F16, tag="oth")

            # ---- phase E: encoders ----
            for a in range(A):
                ab = slice(a * BN, (a + 1) * BN)
                ah = slice(a * H, (a + 1) * H)
                p1 = lxp.tile([H, 2 * BN], F32, tag="lx")
                nc.tensor.matmul(p1[:, 0:BN], t_enc_s[:, ah], xs_t[:, ab],
                                 start=True, stop=False)
                nc.tensor.matmul(p1[:, 0:BN], t_enc_a[:, ah], xa_t[:, ab],
                                 start=False, stop=True)
                nc.tensor.matmul(p1[:, BN:], t_senc[:, ah], xs_t[:, ab])
                sact(sa_t[:, ab], p1[:, 0:BN], AF.Lrelu,
                                     bias=t_benc[:, a:a + 1], scale=1.0,
                                     alpha=0.01)
                sact(s_t[:, ab], p1[:, BN:], AF.Lrelu,
                                     bias=t_bsenc[:, a:a + 1], scale=1.0,
                                     alpha=0.01)
                p3 = ps.tile([H, BN], F32, tag="ps")
                nc.tensor.matmul(p3[:], t_key, sa_t[:, ab])
                nc.vector.tensor_copy(K_t[:, ab], p3[:])
                p4 = lxp.tile([H, 2 * BN], F32, tag="lx")
                nc.tensor.matmul(p4[:, 0:BN], t_val, sa_t[:, ab])
                nc.tensor.matmul(p4[:, BN:], t_sel, s_t[:, ab])
                sact(V_t[:, ab], p4[:, 0:BN], AF.Lrelu,
                                     bias=t_bval[:, 0:1], scale=1.0,
                                     alpha=0.01)
                nc.vector.tensor_copy(S_t[:, ab], p4[:, BN:])

            # ---- phase A: attention per agent i (self pair skipped) ----
            for i in range(A):
                ib = slice(i * BN, (i + 1) * BN)
                P = att.tile([H, A * BN], BF16, tag="P")
                for lo, hi in ((0, i), (i + 1, A)):
                    if lo >= hi:
                        continue
                    nj = hi - lo
                    sl = slice(lo * BN, hi * BN)
                    nc.vector.tensor_mul(
                        P[:, sl].rearrange("p (j b) -> p j b", j=nj),
                        K_t[:, sl].rearrange("p (j b) -> p j b", j=nj),
                        S_t[:, ib].unsqueeze(1).broadcast_to([H, nj, BN]))
                e_t = att3.tile([H, A * BN], F16, tag="e")
                js = [j for j in range(A) if j != i]
                for jc in range(0, len(js), 2):
                    pair = js[jc:jc + 2]
                    lx = lxp.tile([H, 2 * BN], F32, tag="lx")
                    if len(pair) == 2 and pair[1] == pair[0] + 1:
                        j0 = pair[0]
                        nc.tensor.matmul(lx[:, 0:BN], t_R,
                                         P[:, j0 * BN:(j0 + 1) * BN])
                        nc.tensor.matmul(lx[:, BN:], t_R,
                                         P[:, (j0 + 1) * BN:(j0 + 2) * BN])
                        sact(e_t[:, j0 * BN:(j0 + 2) * BN], lx[:], AF.Exp,
                             bias=t_neg4[:, 0:1], scale=1.0)
                    else:
                        for k, j in enumerate(pair):
                            nc.tensor.matmul(lx[:, k * BN:(k + 1) * BN], t_R,
                                             P[:, j * BN:(j + 1) * BN])
                            sact(e_t[:, j * BN:(j + 1) * BN],
                                 lx[:, k * BN:(k + 1) * BN], AF.Exp,
                                 bias=t_neg4[:, 0:1], scale=1.0)
                # zero the self hole: the DVE Z-tree (odd i) reads all 8
                # slots, and it keeps every e/W slice initialized
                nc.gpsimd.memset(e_t[:, ib], 0.0)
                W_t = att.tile([H, A * BN], F16, tag="W")
                nc.vector.tensor_mul(W_t[:], e_t[:], V_t[:])
                js = [j for j in range(A) if j != i]
                nm = nmp.tile([H, BN], F32, tag="nm")
                for k, j in enumerate(js):
                    jb = slice(j * BN, (j + 1) * BN)
                    nc.tensor.matmul(nm[:], t_I, W_t[:, jb],
                                     start=(k == 0), stop=(k == len(js) - 1))
                r = sm.tile([H, BN], F32, tag="r")
                if i % 4 == 0:
                    zx = zxp.tile([H, BN], F32, tag="zx")
                    for k, j in enumerate(js):
                        jb = slice(j * BN, (j + 1) * BN)
                        nc.tensor.matmul(zx[:], t_I, e_t[:, jb],
                                         start=(k == 0), stop=(k == len(js) - 1))
                    nc.vector.reciprocal_approx_fast(r[:], zx[:])
                else:
                    t1 = att.tile([H, A * BN // 2], F16, tag="P")
                    nc.vector.tensor_add(t1[:], e_t[:, 0:A * BN // 2],
                                         e_t[:, A * BN // 2:])
                    t2 = sm.tile([H, A * BN // 4], F16, tag="t2")
                    nc.vector.tensor_add(t2[:], t1[:, 0:A * BN // 4],
                                         t1[:, A * BN // 4:])
                    zs = sm.tile([H, BN], F32, tag="zs")
                    nc.vector.tensor_add(zs[:], t2[:, 0:BN], t2[:, BN:])
                    nc.vector.reciprocal_approx_fast(r[:], zs[:])
                nc.vector.tensor_mul(oth_t[:, ib], nm[:], r[:])

            # ---- phase C: critic per agent ----
            for a in range(A):
                ab = slice(a * BN, (a + 1) * BN)
                ah = slice(a * H, (a + 1) * H)
                pc = lxp.tile([H, 2 * BN], F32, tag="lx")
                nc.tensor.matmul(pc[:, 0:BN], t_c1s[:, ah], s_t[:, ab],
                                 start=True, stop=False)
                nc.tensor.matmul(pc[:, 0:BN], t_c1o[:, ah], oth_t[:, ab],
                                 start=False, stop=True)
                h1 = sm.tile([H, BN], F16, tag="h1")
                sact(h1[:], pc[:, 0:BN], AF.Lrelu,
                                     bias=t_bc1[:, a:a + 1], scale=1.0,
                                     alpha=0.01)
                nc.tensor.matmul(pc[:, BN:], t_c2[:, ah], h1[:])
                h2 = sm.tile([H, BN], F16, tag="h2")
                sact(h2[:], pc[:, BN:], AF.Lrelu,
                                     bias=t_bc2[:, a:a + 1], scale=1.0,
                                     alpha=0.01)
                pc3 = ps.tile([1, BN], F32, tag="ps")
                nc.tensor.matmul(pc3[:], t_c3[:, a:a + 1], h2[:])
                qv = sm.tile([1, BN], F32, tag="qv")
                nc.vector.tensor_scalar_add(qv[:], pc3[:],
                                            t_bc3[0:1, a:a + 1])
                nc.sync.dma_start(q[a, b0:b0 + BN], qv[:])

        from concourse.tile import add_dep_helper
        for prev, nxt in zip(act_chain, act_chain[1:]):
            add_dep_helper(nxt.ins, prev.ins, sync=False,
                           reason="group ACT ops to avoid table reloads")

    nc.compile()
    return nc


_NC_CACHE = None


def _get_nc():
    global _NC_CACHE
    if _NC_CACHE is None:
        _NC_CACHE = _build_program()
    return _NC_CACHE


def _prep_shared(inputs):
    f32 = np.float32

    def t(x):
        return np.ascontiguousarray(x, dtype=np.float16)

    enc_W = np.asarray(inputs["enc_W"], f32)     # [A, IDIM, H]
    senc_W = np.asarray(inputs["senc_W"], f32)   # [A, SDIM, H]
    key_W = np.asarray(inputs["key_W"], f32)     # [NH, H, D]
    sel_W = np.asarray(inputs["sel_W"], f32)
    val_W = np.asarray(inputs["val_W"], f32)
    c1_W = np.asarray(inputs["c1_W"], f32)       # [A, 2H, H]
    c2_W = np.asarray(inputs["c2_W"], f32)
    c3_W = np.asarray(inputs["c3_W"], f32)       # [A, H, 1]

    blk = np.zeros((H, H), f32)
    for n in range(NH):
        blk[n * D:(n + 1) * D, n * D:(n + 1) * D] = 1.0 / np.sqrt(D)

    return {
        "w_enc_s": t(enc_W[:, :SDIM, :].transpose(1, 0, 2).reshape(SDIM, A * H)),
        "w_enc_a": t(enc_W[:, SDIM:, :].transpose(1, 0, 2).reshape(ADIM, A * H)),
        "w_senc": t(senc_W.transpose(1, 0, 2).reshape(SDIM, A * H)),
        "w_key": t(key_W.transpose(1, 0, 2).reshape(H, NH * D)),
        "w_sel": t(sel_W.transpose(1, 0, 2).reshape(H, NH * D)),
        "w_val": t(val_W.transpose(1, 0, 2).reshape(H, NH * D)),
        "w_c1s": t(c1_W[:, :H, :].transpose(1, 0, 2).reshape(H, A * H)),
        "w_c1o": t(c1_W[:, H:, :].transpose(1, 0, 2).reshape(H, A * H)),
        "w_c2": t(c2_W.transpose(1, 0, 2).reshape(H, A * H)),
        "w_c3": t(c3_W[:, :, 0].T),
        "cR": t(blk),
        "cI": t(np.eye(H, dtype=f32)),
        "b_enc": np.ascontiguousarray(np.asarray(inputs["enc_b"], f32).T),
        "b_senc": np.ascontiguousarray(np.asarray(inputs["senc_b"], f32).T),
        "b_val": np.asarray(inputs["val_b"], f32).reshape(NH * D, 1).copy(),
        "b_c1": np.ascontiguousarray(np.asarray(inputs["c1_b"], f32).T),
        "b_c2": np.ascontiguousarray(np.asarray(inputs["c2_b"], f32).T),
        "b_c3": np.ascontiguousarray(np.asarray(inputs["c3_b"], f32).reshape(A, 1).T),
    }


def _run(inputs, trace=False, tmpdir=None):
    from concourse.bass_utils import run_bass_kernel_spmd

    nc = _get_nc()
    shared = _prep_shared(inputs)

    states = np.asarray(inputs["states"], np.float32)   # [A, B, SDIM]
    actions = np.asarray(inputs["actions"], np.float32)
    xs_all = np.ascontiguousarray(
        states.transpose(0, 2, 1), dtype=np.float16)    # [A, SDIM, B]
    xa_all = np.ascontiguousarray(
        actions.transpose(0, 2, 1), dtype=np.float16)   # [A, ADIM, B]

    in_maps = []
    for m in range(M):
        sl = slice(m * BS, (m + 1) * BS)
        im = dict(shared)
        im["xs"] = np.ascontiguousarray(xs_all[:, :, sl])
        im["xa"] = np.ascontiguousarray(xa_all[:, :, sl])
        in_maps.append(im)

    kw = {}
    if trace:
        kw = dict(trace=True, tmpdir=tmpdir)
    br = run_bass_kernel_spmd(nc, in_maps, core_ids=list(range(M)), **kw)
    outs = [br.results[m]["q"] for m in range(M)]       # each [A, BS]
    full = np.concatenate(outs, axis=1)                 # [A, B]
    return full.reshape(A, B, 1).astype(np.float32), br


def kernel(**inputs):
    out, _ = _run(inputs)
    return out


# revision 41
# speedup vs baseline: 1.2079x; 1.2079x over previous
"""AttentionCritic Trainium2 kernel — full inputs in, full output out.

Data-parallel over batch B across 8 NeuronCores (Bs = B/8 = 4096 per core),
params replicated. Per core the forward runs as one Bass/Tile program in
feature-major fp16 layout (host pre-transposes inputs, packs weights).

Per 512-sample tile, three phases (grouped so the ScalarE activation
table switches at most ~3x per tile — table loads are ~1.3us each):
  E: per-agent encoders + K/V/S head projections (PE matmuls, ACT Lrelu,
     DVE PSUM evacuations)
  A: per-sample agent-attention: DVE pair products against a broadcast
     AP, PE block-ones reduce to expanded logits (1/sqrt(D) folded in),
     ACT exp (input shift -4 for fp16 range; self pair killed with a
     -25 shift instead of a mask), numerator and denominator via PE
     identity-matmul PSUM accumulation (denominator on DVE add-tree for
     odd agents to balance engines), DVE fast reciprocal + normalize
  C: per-agent critic MLP; q bias-add on DVE (keeps ACT on Lrelu only)
"""

import numpy as np

A = 8
B = 32768
SDIM = 128
ADIM = 32
H = 128
NH = 4
D = H // NH
M = 8            # cores
BS = B // M      # 4096 batch per core
BN = 512         # batch tile
NT = BS // BN    # 8 tiles


def _build_program():
    from contextlib import ExitStack
    import concourse.bacc as bacc
    import concourse.tile as tile
    import concourse.mybir as mybir

    F16 = mybir.dt.float16
    BF16 = mybir.dt.bfloat16
    F32 = mybir.dt.float32
    AF = mybir.ActivationFunctionType

    nc = bacc.Bacc("TRN2", target_bir_lowering=False, debug=False, num_devices=M)

    def din(name, shape, dt=F16):
        return nc.dram_tensor(name, shape, dt, kind="ExternalInput").ap()

    xs = din("xs", [A, SDIM, BS])              # states^T per agent
    xa = din("xa", [A, ADIM, BS])              # actions^T per agent
    w_enc_s = din("w_enc_s", [SDIM, A * H])
    w_enc_a = din("w_enc_a", [ADIM, A * H])
    w_senc = din("w_senc", [SDIM, A * H])
    w_key = din("w_key", [H, H])
    w_sel = din("w_sel", [H, H])
    w_val = din("w_val", [H, H])
    w_c1s = din("w_c1s", [H, A * H])
    w_c1o = din("w_c1o", [H, A * H])
    w_c2 = din("w_c2", [H, A * H])
    w_c3 = din("w_c3", [H, A])
    cR = din("cR", [H, H])                     # block-ones / sqrt(D)
    cI = din("cI", [H, H])                     # identity
    b_enc = din("b_enc", [H, A], F32)
    b_senc = din("b_senc", [H, A], F32)
    b_val = din("b_val", [H, 1], F32)
    b_c1 = din("b_c1", [H, A], F32)
    b_c2 = din("b_c2", [H, A], F32)
    b_c3 = din("b_c3", [1, A], F32)
    q = nc.dram_tensor("q", [A, BS], F32, kind="ExternalOutput").ap()

    with tile.TileContext(nc) as tc, ExitStack() as ctx:
        wp = ctx.enter_context(tc.tile_pool(name="w", bufs=1))
        io = ctx.enter_context(tc.tile_pool(name="io", bufs=2))
        io1 = ctx.enter_context(tc.tile_pool(name="io1", bufs=1))
        act = ctx.enter_context(tc.tile_pool(name="act", bufs=2))
        att = ctx.enter_context(tc.tile_pool(name="att", bufs=2))
        sm = ctx.enter_context(tc.tile_pool(name="sm", bufs=2))
        ps = ctx.enter_context(tc.tile_pool(name="ps", bufs=1, space="PSUM"))
        lxp = ctx.enter_context(tc.tile_pool(name="lxp", bufs=2, space="PSUM"))
        nmp = ctx.enter_context(tc.tile_pool(name="nmp", bufs=2, space="PSUM"))
        zxp = ctx.enter_context(tc.tile_pool(name="zxp", bufs=1, space="PSUM"))

        def wtile(ap_, shape, dt=F16):
            # weight loads ride the gpsimd DMA queue so the first input
            # tile's sync-queue DMA isn't serialized behind them
            t = wp.tile(shape, dt, tag=ap_.name)
            nc.gpsimd.dma_start(t[:], ap_)
            return t

        t_enc_s = wtile(w_enc_s, [SDIM, A * H])
        t_enc_a = wtile(w_enc_a, [ADIM, A * H])
        t_senc = wtile(w_senc, [SDIM, A * H])
        t_key = wtile(w_key, [H, H])
        t_sel = wtile(w_sel, [H, H])
        t_val = wtile(w_val, [H, H])
        t_c1s = wtile(w_c1s, [H, A * H])
        t_c1o = wtile(w_c1o, [H, A * H])
        t_c2 = wtile(w_c2, [H, A * H])
        t_c3 = wtile(w_c3, [H, A])
        t_R = wtile(cR, [H, H], BF16)
        t_I = wtile(cI, [H, H])
        t_benc = wtile(b_enc, [H, A], F32)
        t_bsenc = wtile(b_senc, [H, A], F32)
        t_bval = wtile(b_val, [H, 1], F32)
        t_bc1 = wtile(b_c1, [H, A], F32)
        t_bc2 = wtile(b_c2, [H, A], F32)
        t_bc3 = wtile(b_c3, [1, A], F32)
        t_neg4 = wp.tile([H, 1], F32, tag="neg4")
        nc.gpsimd.memset(t_neg4[:], -4.0)
        t_neg25 = wp.tile([H, 1], F32, tag="neg25")
        nc.gpsimd.memset(t_neg25[:], -25.0)

        act_chain = []

        def sact(*args, **kw):
            act_chain.append(nc.scalar.activation(*args, **kw))

        for t in range(NT):
            b0 = t * BN
            # ---- input tiles: [feat, (agent, b)] ----
            xs_t = io.tile([SDIM, A * BN], F16, tag="xs")
            for g4 in range(4):
                ga = slice(2 * g4, 2 * g4 + 2)
                nc.sync.dma_start(
                    xs_t[:, 2 * g4 * BN:(2 * g4 + 2) * BN]
                    .rearrange("p (a b) -> p a b", a=2),
                    xs[ga, :, b0:b0 + BN].rearrange("a p b -> p a b"))
            xa_t = io1.tile([ADIM, A * BN], F16, tag="xa")
            nc.sync.dma_start(
                xa_t[:].rearrange("p (a b) -> p a b", a=A),
                xa[:, :, b0:b0 + BN].rearrange("a p b -> p a b"))

            sa_t = io1.tile([H, A * BN], F16, tag="sa")
            s_t = act.tile([H, A * BN], F16, tag="s")
            K_t = act.tile([H, A * BN], F16, tag="K")
            V_t = act.tile([H, A * BN], F16, tag="V")
            S_t = act.tile([H, A * BN], F16, tag="S")
            oth_t = act.tile([H, A * BN], F16, tag="oth")

            # ---- phase E: encoders ----
            for a in range(A):
                ab = slice(a * BN, (a + 1) * BN)
                ah = slice(a * H, (a + 1) * H)
                p1 = lxp.tile([H, 2 * BN], F32, tag="lx")
                nc.tensor.matmul(p1[:, 0:BN], t_enc_s[:, ah], xs_t[:, ab],
                                 start=True, stop=False)
                nc.tensor.matmul(p1[:, 0:BN], t_enc_a[:, ah], xa_t[:, ab],
                                 start=False, stop=True)
                nc.tensor.matmul(p1[:, BN:], t_senc[:, ah], xs_t[:, ab])
                sact(sa_t[:, ab], p1[:, 0:BN], AF.Lrelu,
                                     bias=t_benc[:, a:a + 1], scale=1.0,
                                     alpha=0.01)
                sact(s_t[:, ab], p1[:, BN:], AF.Lrelu,
                                     bias=t_bsenc[:, a:a + 1], scale=1.0,
                                     alpha=0.01)
                p3 = ps.tile([H, BN], F32, tag="ps")
                nc.tensor.matmul(p3[:], t_key, sa_t[:, ab])
                nc.vector.tensor_copy(K_t[:, ab], p3[:])
                p4 = lxp.tile([H, 2 * BN], F32, tag="lx")
                nc.tensor.matmul(p4[:, 0:BN], t_val, sa_t[:, ab])
                nc.tensor.matmul(p4[:, BN:], t_sel, s_t[:, ab])
                sact(V_t[:, ab], p4[:, 0:BN], AF.Lrelu,
                                     bias=t_bval[:, 0:1], scale=1.0,
                                     alpha=0.01)
                nc.vector.tensor_copy(S_t[:, ab], p4[:, BN:])

            # ---- phase A: attention per agent i (self pair skipped) ----
            for i in range(A):
                ib = slice(i * BN, (i + 1) * BN)
                P = att.tile([H, A * BN], BF16, tag="P")
                for lo, hi in ((0, i), (i + 1, A)):
                    if lo >= hi:
                        continue
                    nj = hi - lo
                    sl = slice(lo * BN, hi * BN)
                    nc.vector.tensor_mul(
                        P[:, sl].rearrange("p (j b) -> p j b", j=nj),
                        K_t[:, sl].rearrange("p (j b) -> p j b", j=nj),
                        S_t[:, ib].unsqueeze(1).broadcast_to([H, nj, BN]))
                e_t = att.tile([H, A * BN], F16, tag="e")
                js = [j for j in range(A) if j != i]
                for jc in range(0, len(js), 2):
                    pair = js[jc:jc + 2]
                    lx = lxp.tile([H, 2 * BN], F32, tag="lx")
                    if len(pair) == 2 and pair[1] == pair[0] + 1:
                        j0 = pair[0]
                        nc.tensor.matmul(lx[:, 0:BN], t_R,
                                         P[:, j0 * BN:(j0 + 1) * BN])
                        nc.tensor.matmul(lx[:, BN:], t_R,
                                         P[:, (j0 + 1) * BN:(j0 + 2) * BN])
                        sact(e_t[:, j0 * BN:(j0 + 2) * BN], lx[:], AF.Exp,
                             bias=t_neg4[:, 0:1], scale=1.0)
                    else:
                        for k, j in enumerate(pair):
                            nc.tensor.matmul(lx[:, k * BN:(k + 1) * BN], t_R,
                                             P[:, j * BN:(j + 1) * BN])
                            sact(e_t[:, j * BN:(j + 1) * BN],
                                 lx[:, k * BN:(k + 1) * BN], AF.Exp,
                                 bias=t_neg4[:, 0:1], scale=1.0)
                # zero the self hole: the DVE Z-tree (odd i) reads all 8
                # slots, and it keeps every e/W slice initialized
                nc.gpsimd.memset(e_t[:, ib], 0.0)
                W_t = att.tile([H, A * BN], F16, tag="W")
                nc.vector.tensor_mul(W_t[:], e_t[:], V_t[:])
                js = [j for j in range(A) if j != i]
                nm = nmp.tile([H, BN], F32, tag="nm")
                for k, j in enumerate(js):
                    jb = slice(j * BN, (j + 1) * BN)
                    nc.tensor.matmul(nm[:], t_I, W_t[:, jb],
                                     start=(k == 0), stop=(k == len(js) - 1))
                r = sm.tile([H, BN], F32, tag="r")
                if i % 4 == 0:
                    zx = zxp.tile([H, BN], F32, tag="zx")
                    for k, j in enumerate(js):
                        jb = slice(j * BN, (j + 1) * BN)
                        nc.tensor.matmul(zx[:], t_I, e_t[:, jb],
                                         start=(k == 0), stop=(k == len(js) - 1))
                    nc.vector.reciprocal_approx_fast(r[:], zx[:])
                else:
                    t1 = att.tile([H, A * BN // 2], F16, tag="t1")
                    nc.vector.tensor_add(t1[:], e_t[:, 0:A * BN // 2],
                                         e_t[:, A * BN // 2:])
                    t2 = sm.tile([H, A * BN // 4], F16, tag="t2")
                    nc.vector.tensor_add(t2[:], t1[:, 0:A * BN // 4],
                                         t1[:, A * BN // 4:])
                    zs = sm.tile([H, BN], F32, tag="zs")
                    nc.vector.tensor_add(zs[:], t2[:, 0:BN], t2[:, BN:])
                    nc.vector.reciprocal_approx_fast(r[:], zs[:])
                nc.vector.tensor_mul(oth_t[:, ib], nm[:], r[:])

            # ---- phase C: critic per agent ----
            for a in range(A):
                ab = slice(a * BN, (a + 1) * BN)
                ah = slice(a * H, (a + 1) * H)
                pc = lxp.tile([H, 2 * BN], F32, tag="lx")
                nc.tensor.matmul(pc[:, 0:BN], t_c1s[:, ah], s_t[:, ab],
                                 start=True, stop=False)
                nc.tensor.matmul(pc[:, 0:BN], t_c1o[:, ah], oth_t[:, ab],
                                 start=False, stop=True)
                h1 = sm.tile([H, BN], F16, tag="h1")
                sact(h1[:], pc[:, 0:BN], AF.Lrelu,
                                     bias=t_bc1[:, a:a + 1], scale=1.0,
                                     alpha=0.01)
                nc.tensor.matmul(pc[:, BN:], t_c2[:, ah], h1[:])
                h2 = sm.tile([H, BN], F16, tag="h2")
                sact(h2[:], pc[:, BN:], AF.Lrelu,
                                     bias=t_bc2[:, a:a + 1], scale=1.0,
                                     alpha=0.01)
                pc3 = ps.tile([1, BN], F32, tag="ps")
                nc.tensor.matmul(pc3[:], t_c3[:, a:a + 1], h2[:])
                qv = sm.tile([1, BN], F32, tag="qv")
                nc.vector.tensor_scalar_add(qv[:], pc3[:],
                                            t_bc3[0:1, a:a + 1])
                nc.sync.dma_start(q[a, b0:b0 + BN], qv[:])

        from concourse.tile import add_dep_helper
        for prev, nxt in zip(act_chain, act_chain[1:]):
            add_dep_helper(nxt.ins, prev.ins, sync=False,
                           reason="group ACT ops to avoid table reloads")

    nc.compile()
    return nc


_NC_CACHE = None


def _get_nc():
    global _NC_CACHE
    if _NC_CACHE is None:
        _NC_CACHE = _build_program()
    return _NC_CACHE


def _prep_shared(inputs):
    f32 = np.float32

    def t(x):
        return np.ascontiguousarray(x, dtype=np.float16)

    enc_W = np.asarray(inputs["enc_W"], f32)     # [A, IDIM, H]
    senc_W = np.asarray(inputs["senc_W"], f32)   # [A, SDIM, H]
    key_W = np.asarray(inputs["key_W"], f32)     # [NH, H, D]
    sel_W = np.asarray(inputs["sel_W"], f32)
    val_W = np.asarray(inputs["val_W"], f32)
    c1_W = np.asarray(inputs["c1_W"], f32)       # [A, 2H, H]
    c2_W = np.asarray(inputs["c2_W"], f32)
    c3_W = np.asarray(inputs["c3_W"], f32)       # [A, H, 1]

    blk = np.zeros((H, H), f32)
    for n in range(NH):
        blk[n * D:(n + 1) * D, n * D:(n + 1) * D] = 1.0 / np.sqrt(D)

    return {
        "w_enc_s": t(enc_W[:, :SDIM, :].transpose(1, 0, 2).reshape(SDIM, A * H)),
        "w_enc_a": t(enc_W[:, SDIM:, :].transpose(1, 0, 2).reshape(ADIM, A * H)),
        "w_senc": t(senc_W.transpose(1, 0, 2).reshape(SDIM, A * H)),
        "w_key": t(key_W.transpose(1, 0, 2).reshape(H, NH * D)),
        "w_sel": t(sel_W.transpose(1, 0, 2).reshape(H, NH * D)),
        "w_val": t(val_W.transpose(1, 0, 2).reshape(H, NH * D)),
        "w_c1s": t(c1_W[:, :H, :].transpose(1, 0, 2).reshape(H, A * H)),
        "w_c1o": t(c1_W[:, H:, :].transpose(1, 0, 2).reshape(H, A * H)),
        "w_c2": t(c2_W.transpose(1, 0, 2).reshape(H, A * H)),
        "w_c3": t(c3_W[:, :, 0].T),
        "cR": t(blk),
        "cI": t(np.eye(H, dtype=f32)),
        "b_enc": np.ascontiguousarray(np.asarray(inputs["enc_b"], f32).T),
        "b_senc": np.ascontiguousarray(np.asarray(inputs["senc_b"], f32).T),
        "b_val": np.asarray(inputs["val_b"], f32).reshape(NH * D, 1).copy(),
        "b_c1": np.ascontiguousarray(np.asarray(inputs["c1_b"], f32).T),
        "b_c2": np.ascontiguousarray(np.asarray(inputs["c2_b"], f32).T),
        "b_c3": np.ascontiguousarray(np.asarray(inputs["c3_b"], f32).reshape(A, 1).T),
    }


def _run(inputs, trace=False, tmpdir=None):
    from concourse.bass_utils import run_bass_kernel_spmd

    nc = _get_nc()
    shared = _prep_shared(inputs)

    states = np.asarray(inputs["states"], np.float32)   # [A, B, SDIM]
    actions = np.asarray(inputs["actions"], np.float32)
    xs_all = np.ascontiguousarray(
        states.transpose(0, 2, 1), dtype=np.float16)    # [A, SDIM, B]
    xa_all = np.ascontiguousarray(
        actions.transpose(0, 2, 1), dtype=np.float16)   # [A, ADIM, B]

    in_maps = []
    for m in range(M):
        sl = slice(m * BS, (m + 1) * BS)
        im = dict(shared)
        im["xs"] = np.ascontiguousarray(xs_all[:, :, sl])
        im["xa"] = np.ascontiguousarray(xa_all[:, :, sl])
        in_maps.append(im)

    kw = {}
    if trace:
        kw = dict(trace=True, tmpdir=tmpdir)
    br = run_bass_kernel_spmd(nc, in_maps, core_ids=list(range(M)), **kw)
    outs = [br.results[m]["q"] for m in range(M)]       # each [A, BS]
    full = np.concatenate(outs, axis=1)                 # [A, B]
    return full.reshape(A, B, 1).astype(np.float32), br


def kernel(**inputs):
    out, _ = _run(inputs)
    return out
